# revision 13
# baseline (speedup 1.0000x reference)
"""Trainium2 Bass kernel for nn_Backbone_SPCONV (8-layer masked sparse-conv
backbone) on 8 NeuronCores, SPMD batch(2) x depth-slab(4) sharding.

Executed as 4 SPMD launches (one per 2-conv+pool module); the host reshards
pooled outputs (halo + padding + kw-packing) between launches.  Inside a
launch: channel-major conv matmuls psum[co,N] accumulating over (kd,kh) with
kw folded into the contraction dim (activations stored as 3 w-shifted copies)
for Cin<=64; masked-BN stats are per-core partial sums AllReduced over the 8
cores after each conv layer; relu(a*y+b)*mask applied at plane load."""
import sys
import numpy as np

sys.path.insert(0, "/opt/trn_rl_repo")
import concourse.bass as bass
import concourse.bacc as bacc
import concourse.mybir as mybir
import concourse.tile as tile
from concourse.bass_utils import run_bass_kernel_spmd

F32 = mybir.dt.float32
BN_EPS = 1e-5
DIMS = [16, 32, 32, 64, 64, 128, 128, 256, 256]
SUBM = [False, True, False, True, False, True, False, True]
B = 2
NC8 = 8
GEO = {64: (66, 4360), 32: (34, 1160), 16: (18, 328), 8: (10, 104)}
BASE = 2
RES = [64, 64, 32, 32, 16, 16, 8, 8]
FOLD = [True, True, True, True, True, False, False, False]
# per-module tables: extents relative to slab s (N_own per core at conv res)
NOWN = [16, 8, 4, 2]  # owned planes per core at module conv res
# x extent: [N*s-3, +N+5); y0 ext: [N*s-2, +N+4); y1 ext: [N*s-1, +N+3)


def wins(res):
    if res == 64:
        return [(8 * t, 8, 64) for t in range(8)]
    if res == 32:
        return [(16 * t, 16, 32) for t in range(2)]
    return [(0, res, res)]


def dilate_np(m):
    D, H, W = m.shape
    p = np.zeros((D + 2, H + 2, W + 2), m.dtype)
    p[1:-1, 1:-1, 1:-1] = m
    out = np.zeros_like(m)
    for a in range(3):
        for b in range(3):
            for c in range(3):
                out = np.maximum(out, p[a:a + D, b:b + H, c:c + W])
    return out


def sumpool_np(x):
    D, H, W = x.shape[-3:]
    p = np.zeros(x.shape[:-3] + (D + 2, H + 2, W + 2), x.dtype)
    p[..., 1:-1, 1:-1, 1:-1] = x
    D2, H2, W2 = D // 2, H // 2, W // 2
    out = np.zeros(x.shape[:-3] + (D2, H2, W2), x.dtype)
    for a in range(3):
        for b in range(3):
            for c in range(3):
                out += p[..., a:a + 2 * D2:2, b:b + 2 * H2:2, c:c + 2 * W2:2]
    return out


def pad_flat(m, res):
    PW, al = GEO[res]
    P = m.shape[0]
    pp = np.zeros((P, PW, PW), np.float32)
    pp[:, 1:-1, 1:-1] = m
    out = np.zeros((P, al), np.float32)
    out[:, BASE:BASE + PW * PW] = pp.reshape(P, -1)
    return out


def plane_or(mfull, gp, fill=0.0):
    R = mfull.shape[-1]
    if 0 <= gp < R:
        return np.asarray(mfull[..., gp, :, :], np.float32)
    return np.full(mfull.shape[:-3] + (R, R), fill, np.float32)


def pack_x(xfull, b, s, N, res, fold):
    """Build conv-input array for one core: planes [N*s-3, N*s+N+2), padded,
    kw-packed (3 shifted copies) when fold."""
    C = xfull.shape[1]
    npl = N + 5
    PW, al = GEO[res]
    rows = 3 * C if fold else C
    out = np.zeros((rows, npl, al), np.float32)
    for i in range(npl):
        gp = N * s - 3 + i
        pl = pad_flat(plane_or(xfull[b].transpose(1, 0, 2, 3)[None][0], gp)
                      if False else
                      np.stack([plane_or(xfull[b, ch], gp) for ch in range(C)]),
                      res)
        body = pl[:, BASE:BASE + PW * PW]
        if fold:
            for kw in range(3):
                out[kw * C:(kw + 1) * C, i, 3 - kw:3 - kw + PW * PW] = body
        else:
            out[:, i, BASE:BASE + PW * PW] = body
    return out


def build_wspec(l, params):
    cin, cout = DIMS[l], DIMS[l + 1]
    w = np.asarray(params[l][0], np.float32)
    blocks, spec = [], []
    if FOLD[l]:
        kl = 3 * cin
        krs = [(0, kl)] if kl <= 128 else [(0, 128), (128, kl - 128)]
        for kd in range(3):
            for kh in range(3):
                full = np.zeros((kl, cout), np.float32)
                for kw in range(3):
                    full[kw * cin:(kw + 1) * cin] = w[:, :, kd, kh, kw].T
                for (r0, rl) in krs:
                    blocks.append(full[r0:r0 + rl])
                    spec.append((kd, kh, r0, rl, 0, cout, None))
    else:
        for hf in range((cout + 127) // 128):
            co0 = hf * 128
            col = min(128, cout - co0)
            for kd in range(3):
                for kh in range(3):
                    for kw in range(3):
                        for cb in range((cin + 127) // 128):
                            ci0 = cb * 128
                            cil = min(128, cin - ci0)
                            blocks.append(np.ascontiguousarray(
                                w[co0:co0 + col, ci0:ci0 + cil, kd, kh, kw].T))
                            spec.append((kd, kh, ci0, cil, co0, col, kw))
    arr = np.zeros((len(blocks), 128, 128), np.float32)
    for i, bb in enumerate(blocks):
        arr[i, :bb.shape[0], :bb.shape[1]] = bb
    return arr, spec


def rep_ap(t, off, n_part, n_free):
    a = t[:]
    return bass.AP(tensor=a.tensor, offset=off, ap=[[0, n_part], [1, n_free]])


def dram_ap(t, row0, nrows, plane, al, off=0, nfree=None):
    npl = t.shape[1]
    a = t[:]
    return bass.AP(tensor=a.tensor,
                   offset=row0 * npl * al + plane * al + off,
                   ap=[[npl * al, nrows], [1, nfree if nfree else al]])


_MODCACHE = {}
EXEC_NS = []


def build_module(m, inv_n0, inv_n1, specs):
    """Graph for module m: conv l0=2m (input ext), conv l1=2m+1, pool."""
    l0, l1 = 2 * m, 2 * m + 1
    res = RES[l0]
    PW, al = GEO[res]
    r2 = res // 2
    N = NOWN[m]
    cin, cmid, cout = DIMS[l0], DIMS[l0 + 1], DIMS[l1 + 1]
    nc = bacc.Bacc("TRN2", target_bir_lowering=False, debug=False,
                   num_devices=NC8)
    io = {}
    rows_x = 3 * cin if FOLD[l0] else cin
    io["x"] = nc.dram_tensor("x", [rows_x, N + 5, al], F32, kind="ExternalInput")
    for l in (l0, l1):
        nb = len(specs[l][1])
        io[f"wl{l}"] = nc.dram_tensor(f"wl{l}", [nb, 128, 128], F32,
                                      kind="ExternalInput")
        io[f"g{l}"] = nc.dram_tensor(f"g{l}", [DIMS[l + 1], 1], F32,
                                     kind="ExternalInput")
        io[f"sm{l}"] = nc.dram_tensor(f"sm{l}", [N, res * res], F32,
                                      kind="ExternalInput")
    lm_rows = 3 if FOLD[l1] else 1
    io["lm0"] = nc.dram_tensor("lm0", [lm_rows, N + 3, al], F32,
                               kind="ExternalInput")
    io["lm1"] = nc.dram_tensor("lm1", [N + 1, al], F32, kind="ExternalInput")
    io["invc"] = nc.dram_tensor("invc", [max(N // 2, 1), r2 * r2], F32,
                                kind="ExternalInput")
    io["zeros"] = nc.dram_tensor("zeros", [1, 4608], F32, kind="ExternalInput")
    oo = nc.dram_tensor("o", [cout, max(N // 2, 1), r2 * r2], F32,
                        kind="ExternalOutput")
    rows_y0 = 3 * cmid if FOLD[l1] else cmid
    y0 = nc.dram_tensor("y0b", [rows_y0, N + 3, al], F32)
    y1 = nc.dram_tensor("y1b", [cout, N + 1, al], F32)
    arb = {}
    for l in (l0, l1):
        co = DIMS[l + 1]
        wdt = 2 * ((co + 127) // 128)
        arb[l] = (nc.dram_tensor(f"r{l}i", [min(co, 128), wdt], F32),
                  nc.dram_tensor(f"r{l}o", [min(co, 128), wdt], F32,
                                 addr_space="Shared"))
    RG8 = [list(range(8))]
    inv_ns = {l0: inv_n0, l1: inv_n1}
    ab = {}

    with tile.TileContext(nc) as tc:
        with (tc.tile_pool(name="ab", bufs=1) as abp,
              tc.tile_pool(name="ps", bufs=3, space="PSUM") as ps):
            # guard prezero for packed y0 (and y1 tail cols, harmless)
            for t, rows, npl in ((y0, rows_y0, N + 3), (y1, cout, N + 1)):
                for off in (0, al - 4):
                    nc.gpsimd.dma_start(
                        bass.AP(tensor=t[:].tensor, offset=off,
                                ap=[[al, rows * npl], [1, 4]]),
                        bass.AP(tensor=io["zeros"][:].tensor, offset=0,
                                ap=[[0, rows * npl], [1, 4]]))

            def load_xplane(l, idx, cache, srcbuf, src_rows, normalize,
                            lm_name, fold_in, sb, mkp):
                if idx in cache:
                    return cache[idx]
                cl = DIMS[l]
                out = []
                for r0 in range(0, src_rows, 128):
                    rl = min(128, src_rows - r0)
                    t = sb.tile([rl, al], F32, tag=f"xi{l}_{r0}")
                    sap = dram_ap(srcbuf, r0, rl, idx, al)
                    nc.gpsimd.dma_start(t[:], sap)
                    if normalize:
                        av, bv = ab[l - 1]
                        if fold_in:
                            ar = abp.tile([rl, 1], F32, tag=f"ar{l}_{r0}")
                            br = abp.tile([rl, 1], F32, tag=f"br{l}_{r0}")
                            for kw in range(3):
                                lo, hi = kw * cl, (kw + 1) * cl
                                s0, s1 = max(lo, r0), min(hi, r0 + rl)
                                if s0 >= s1:
                                    continue
                                nc.vector.tensor_copy(ar[s0 - r0:s1 - r0],
                                                      av[s0 - lo:s1 - lo, 0:1])
                                nc.vector.tensor_copy(br[s0 - r0:s1 - r0],
                                                      bv[s0 - lo:s1 - lo, 0:1])
                            a_ap, b_ap = ar[:], br[:]
                        else:
                            hfb = r0 // 128
                            a_ap = av[:rl, hfb:hfb + 1]
                            b_ap = bv[:rl, hfb:hfb + 1]
                        nc.scalar.activation(t[:], t[:],
                                             mybir.ActivationFunctionType.Relu,
                                             bias=b_ap, scale=a_ap)
                        lmt = mkp.tile([rl, al], F32, tag=f"lt{l}_{r0}")
                        npl_m = io[lm_name].shape[1] if fold_in else 0
                        if fold_in:
                            for kw in range(3):
                                lo, hi = kw * cl, (kw + 1) * cl
                                s0, s1 = max(lo, r0), min(hi, r0 + rl)
                                if s0 >= s1:
                                    continue
                                nc.gpsimd.dma_start(
                                    lmt[s0 - r0:s1 - r0, :],
                                    rep_ap(io[lm_name],
                                           kw * npl_m * al + idx * al,
                                           s1 - s0, al))
                        else:
                            nc.gpsimd.dma_start(
                                lmt[:], rep_ap(io[lm_name], idx * al, rl, al))
                        nc.vector.tensor_mul(t[:], t[:], lmt[:])
                    out.append((r0, rl, t))
                cache[idx] = out
                return out

            def conv(l, srcbuf, src_rows, normalize, lm_name, n_out, own_lo,
                     ybuf, y_rows, y_fold):
                from contextlib import ExitStack
                ctx = ExitStack()
                sb = ctx.enter_context(tc.tile_pool(name=f"sb{l}", bufs=3))
                wp = ctx.enter_context(tc.tile_pool(name=f"wp{l}", bufs=1))
                mkp = ctx.enter_context(tc.tile_pool(name=f"mk{l}", bufs=2))
                scp = ctx.enter_context(tc.tile_pool(name=f"sc{l}", bufs=3))
                co = DIMS[l + 1]
                spec = specs[l][1]
                nhalf = (co + 127) // 128
                fold_in = FOLD[l]
                wtiles = []
                for bi, sp in enumerate(spec):
                    wt = wp.tile([128, 128], F32, tag=f"w{l}_{bi}")
                    nc.gpsimd.dma_start(wt[:sp[3], :sp[5]],
                                        io[f"wl{l}"][bi, :sp[3], :sp[5]])
                    wtiles.append(wt)
                stt = []
                for hf in range(nhalf):
                    col = min(128, co - hf * 128)
                    st = sb.tile([col, 2], F32, tag=f"st{l}_{hf}")
                    nc.vector.memset(st[:], 0.0)
                    stt.append(st)
                cache = {}
                for i in range(n_out):
                    owned = own_lo <= i < own_lo + N
                    ytiles = []
                    for hf in range(nhalf):
                        col = min(128, co - hf * 128)
                        yt = sb.tile([col, al], F32, tag=f"yo{l}_{hf}")
                        nc.vector.memset(yt[:, 0:BASE + PW + 1], 0.0)
                        nc.vector.memset(yt[:, BASE + (PW - 1) * PW - 1:al], 0.0)
                        yap = yt[:]
                        nc.vector.memset(
                            bass.AP(tensor=yap.tensor,
                                    offset=yap.offset + BASE + 2 * PW - 1,
                                    ap=[list(yap.ap[0]), [PW, PW - 3], [1, 2]]),
                            0.0)
                        ytiles.append(yt)
                    for (h0, nr, ncl) in wins(res):
                        NN = nr * ncl
                        for hf in range(nhalf):
                            col = min(128, co - hf * 128)
                            pt = ps.tile([col, NN], F32, tag=f"pp{hf}")
                            mms = [k for k, sp in enumerate(spec)
                                   if sp[4] == hf * 128]
                            for mi, k in enumerate(mms):
                                kd, kh, r0, rl, _, _, kw = spec[k]
                                xt = load_xplane(l, i + kd, cache, srcbuf,
                                                 src_rows, normalize, lm_name,
                                                 fold_in, sb, mkp)
                                for (tr0, trl, tt) in xt:
                                    if tr0 <= r0 < tr0 + trl:
                                        break
                                bb = tt[r0 - tr0:r0 - tr0 + rl, :]
                                woff = 1 if kw is None else kw
                                rhs = bass.AP(
                                    tensor=bb.tensor,
                                    offset=bb.offset + BASE + (h0 + kh) * PW + woff,
                                    ap=[list(bb.ap[0]), [PW, nr], [1, ncl]])
                                nc.tensor.matmul(pt[:], wtiles[k][:rl, :spec[k][5]],
                                                 rhs, start=(mi == 0),
                                                 stop=(mi == len(mms) - 1))
                            if owned:
                                mkt = mkp.tile([col, NN], F32, tag="mk")
                                nc.gpsimd.dma_start(
                                    mkt[:],
                                    rep_ap(io[f"sm{l}"],
                                           (i - own_lo) * res * res + h0 * res,
                                           col, NN))
                                t2 = scp.tile([col, NN], F32, tag="t2")
                                nc.vector.tensor_mul(t2[:], pt[:], mkt[:])
                                r1 = scp.tile([col, 1], F32, tag="r1")
                                nc.vector.reduce_sum(r1[:], t2[:],
                                                     axis=mybir.AxisListType.X)
                                nc.vector.tensor_add(stt[hf][:, 0:1],
                                                     stt[hf][:, 0:1], r1[:])
                                nc.vector.tensor_mul(t2[:], t2[:], t2[:])
                                nc.vector.reduce_sum(r1[:], t2[:],
                                                     axis=mybir.AxisListType.X)
                                nc.vector.tensor_add(stt[hf][:, 1:2],
                                                     stt[hf][:, 1:2], r1[:])
                            yap = ytiles[hf][:]
                            ydst = bass.AP(
                                tensor=yap.tensor,
                                offset=yap.offset + BASE + (h0 + 1) * PW + 1,
                                ap=[list(yap.ap[0]), [PW, nr], [1, ncl]])
                            nc.scalar.activation(ydst, pt[:],
                                                 mybir.ActivationFunctionType.Copy)
                    for hf in range(nhalf):
                        col = min(128, co - hf * 128)
                        if y_fold:
                            for kw in range(3):
                                nc.gpsimd.dma_start(
                                    dram_ap(ybuf, kw * co, col, i, al,
                                            off=3 - kw, nfree=PW * PW),
                                    ytiles[hf][:, BASE:BASE + PW * PW])
                        else:
                            nc.gpsimd.dma_start(
                                dram_ap(ybuf, hf * 128, col, i, al),
                                ytiles[hf][:])
                # stats AllReduce -> a,b
                ari, aro = arb[l]
                for hf in range(nhalf):
                    nc.gpsimd.dma_start(ari[:stt[hf].shape[0], 2 * hf:2 * hf + 2],
                                        stt[hf][:])
                nc.gpsimd.collective_compute("AllReduce", mybir.AluOpType.add,
                                             replica_groups=RG8,
                                             ins=[ari[:]], outs=[aro[:]])
                S = sb.tile(list(aro.shape), F32, tag=f"S{l}")
                nc.gpsimd.dma_start(S[:], aro[:])
                gt = sb.tile([min(co, 128), nhalf], F32, tag=f"gt{l}")
                for hf in range(nhalf):
                    colh = min(128, co - hf * 128)
                    nc.gpsimd.dma_start(gt[:colh, hf:hf + 1],
                                        io[f"g{l}"][hf * 128:hf * 128 + colh, :])
                av = abp.tile([min(co, 128), nhalf], F32, tag=f"av{l}")
                bv = abp.tile([min(co, 128), nhalf], F32, tag=f"bv{l}")
                for hf in range(nhalf):
                    col = min(128, co - hf * 128)
                    mean = scp.tile([col, 1], F32, tag="mean")
                    var = scp.tile([col, 1], F32, tag="var")
                    msq = scp.tile([col, 1], F32, tag="msq")
                    nc.vector.tensor_scalar_mul(mean[:],
                                                S[:col, 2 * hf:2 * hf + 1],
                                                inv_ns[l])
                    nc.vector.tensor_scalar_mul(var[:],
                                                S[:col, 2 * hf + 1:2 * hf + 2],
                                                inv_ns[l])
                    nc.vector.tensor_mul(msq[:], mean[:], mean[:])
                    nc.vector.tensor_sub(var[:], var[:], msq[:])
                    nc.vector.tensor_scalar_add(var[:], var[:], BN_EPS)
                    sd = scp.tile([col, 1], F32, tag="sd")
                    nc.scalar.activation(sd[:], var[:],
                                         mybir.ActivationFunctionType.Sqrt)
                    rsd = scp.tile([col, 1], F32, tag="rsd")
                    nc.vector.reciprocal(rsd[:], sd[:])
                    nc.vector.tensor_mul(av[:col, hf:hf + 1],
                                         gt[:col, hf:hf + 1], rsd[:])
                    nc.vector.tensor_mul(msq[:], av[:col, hf:hf + 1], mean[:])
                    nc.vector.tensor_scalar_mul(bv[:col, hf:hf + 1],
                                                msq[:], -1.0)
                ab[l] = (av, bv)
                ctx.close()

            conv(l0, io["x"], rows_x, False, None, N + 3, 2, y0, rows_y0,
                 FOLD[l1])
            conv(l1, y0, rows_y0, True, "lm0", N + 1, 1, y1, cout, False)
            # pool
            from contextlib import ExitStack
            pctx = ExitStack()
            sb = pctx.enter_context(tc.tile_pool(name="sbp", bufs=3))
            mkp = pctx.enter_context(tc.tile_pool(name="mkp", bufs=2))
            scp = pctx.enter_context(tc.tile_pool(name="scp", bufs=3))
            n_own2 = max(N // 2, 1)
            nhalf = (cout + 127) // 128
            pcache = {}
            for p in range(n_own2):
                for hf in range(nhalf):
                    col = min(128, cout - hf * 128)
                    accT = scp.tile([col, r2 * r2], F32, tag=f"pa{hf}")
                    for kd in range(3):
                        idx = 2 * p + kd
                        key = (hf, idx)
                        if key not in pcache:
                            xe = sb.tile([col, al], F32, tag=f"px{hf}")
                            nc.gpsimd.dma_start(
                                xe[:], dram_ap(y1, hf * 128, col, idx, al))
                            av, bv = ab[l1]
                            nc.scalar.activation(
                                xe[:], xe[:],
                                mybir.ActivationFunctionType.Relu,
                                bias=bv[:col, hf:hf + 1],
                                scale=av[:col, hf:hf + 1])
                            lmt = mkp.tile([col, al], F32, tag=f"plm{hf}")
                            nc.gpsimd.dma_start(
                                lmt[:], rep_ap(io["lm1"], idx * al, col, al))
                            nc.vector.tensor_mul(xe[:], xe[:], lmt[:])
                            u = scp.tile([col, PW * r2], F32, tag=f"pu{hf}")
                            xa = xe[:]

                            def wap(o):
                                return bass.AP(tensor=xa.tensor,
                                               offset=xa.offset + BASE + o,
                                               ap=[list(xa.ap[0]), [PW, PW],
                                                   [2, r2]])
                            nc.vector.tensor_add(u[:], wap(0), wap(1))
                            nc.vector.tensor_add(u[:], u[:], wap(2))
                            v = scp.tile([col, r2 * r2], F32, tag=f"pv{hf}_{idx % 3}")
                            ua = u[:]

                            def hap(o):
                                return bass.AP(tensor=ua.tensor,
                                               offset=ua.offset + o * r2,
                                               ap=[list(ua.ap[0]), [2 * r2, r2],
                                                   [1, r2]])
                            nc.vector.tensor_add(v[:], hap(0), hap(1))
                            nc.vector.tensor_add(v[:], v[:], hap(2))
                            pcache[key] = v
                        v = pcache[key]
                        if kd == 0:
                            nc.vector.tensor_copy(accT[:], v[:])
                        else:
                            nc.vector.tensor_add(accT[:], accT[:], v[:])
                    ivt = mkp.tile([col, r2 * r2], F32, tag=f"iv{hf}")
                    nc.gpsimd.dma_start(
                        ivt[:], rep_ap(io["invc"], p * r2 * r2, col, r2 * r2))
                    nc.vector.tensor_mul(accT[:], accT[:], ivt[:])
                    nc.gpsimd.dma_start(
                        dram_ap(oo, hf * 128, col, p, r2 * r2), accT[:])
            pctx.close()
    nc.compile()
    return nc


def kernel(feats, occ, params):
    feats = np.asarray(feats, np.float32)
    occ = np.asarray(occ)
    m0 = (occ[:, 0] == 0).astype(np.float32)
    m = m0
    pm, cnts = [], []
    for l in range(8):
        if not SUBM[l]:
            m = np.stack([dilate_np(m[b]) for b in range(B)])
        pm.append(m)
        if l % 2 == 1:
            cnt = sumpool_np(m)
            cnts.append(cnt)
            m = (cnt > 0).astype(np.float32)
    inv_n = [1.0 / max(float(pm[l].sum()), 1.0) for l in range(8)]
    specs = {l: build_wspec(l, params) for l in range(8)}

    x_cur = feats * m0[:, None]  # full-grid input to module 0
    outs_np = []
    for mo in range(4):
        l0, l1 = 2 * mo, 2 * mo + 1
        res = RES[l0]
        PW, al = GEO[res]
        N = NOWN[mo]
        r2 = res // 2
        n_own2 = max(N // 2, 1)
        key = mo
        if key not in _MODCACHE:
            _MODCACHE[key] = build_module(mo, inv_n[l0], inv_n[l1], specs)
        nc = _MODCACHE[key]
        ins = []
        for c in range(NC8):
            b, s = c // 4, c % 4
            d = {"x": pack_x(x_cur, b, s, N, res, FOLD[l0]),
                 "zeros": np.zeros((1, 4608), np.float32)}
            for l in (l0, l1):
                d[f"wl{l}"] = specs[l][0]
                d[f"g{l}"] = np.asarray(params[l][1], np.float32).reshape(-1, 1)
                lo = N * s
                d[f"sm{l}"] = pm[l][b][lo:lo + N].reshape(N, -1).astype(np.float32)
            # lm0: mask pm[l0] over y0 extent [N*s-2, +N+4), packed if FOLD[l1]
            g0 = N * s - 2
            if FOLD[l1]:
                a0 = np.zeros((3, N + 3, al), np.float32)
                for i in range(N + 3):
                    p = pad_flat(plane_or(pm[l0][b], g0 + i)[None], res)[0]
                    for kw in range(3):
                        a0[kw, i, 3 - kw:3 - kw + PW * PW] = p[BASE:BASE + PW * PW]
            else:
                a0 = np.zeros((1, N + 3, al), np.float32)
                for i in range(N + 3):
                    a0[0, i] = pad_flat(plane_or(pm[l0][b], g0 + i)[None], res)[0]
            d["lm0"] = a0
            g1 = N * s - 1
            a1 = np.zeros((N + 1, al), np.float32)
            for i in range(N + 1):
                a1[i] = pad_flat(plane_or(pm[l1][b], g1 + i)[None], res)[0]
            d["lm1"] = a1
            lo2 = n_own2 * s
            cnt = cnts[mo][b][lo2:lo2 + n_own2].reshape(n_own2, -1)
            d["invc"] = (1.0 / np.maximum(cnt, 1.0)).astype(np.float32)
            ins.append(d)
        import os, time as _t
        _tr = os.environ.get("BASS_KERNEL_TRACE") == "1"
        _t0 = _t.time()
        try:
            res_run = run_bass_kernel_spmd(nc, ins, list(range(NC8)), trace=_tr)
        except Exception:
            res_run = run_bass_kernel_spmd(nc, ins, list(range(NC8)))
        _wall = (_t.time() - _t0) * 1e9
        EXEC_NS.append(res_run.exec_time_ns if res_run.exec_time_ns
                       else _wall)
        cout = DIMS[l1 + 1]
        full = np.zeros((B, cout, r2, r2, r2), np.float32)
        for c in range(NC8):
            b, s = c // 4, c % 4
            lo2 = n_own2 * s
            full[b, :, lo2:lo2 + n_own2] = \
                res_run.results[c]["o"].reshape(cout, n_own2, r2, r2)
        outs_np.append(full)
        x_cur = full
    return tuple(outs_np)
